# revision 3
# baseline (speedup 1.0000x reference)
"""Trainium2 Bass kernel for AfmoeMoE: token-choice top-2 MoE + shared expert.

Sharding (8 cores):
  - Routed experts: expert-parallel. Core c owns expert c's GLU-MLP weights;
    the host routes tokens (router math replicated on CPU), gathers each
    expert's tokens into a fixed-capacity buffer, and scatter-adds results.
  - Shared expert: tensor-parallel over FS in halves x data-parallel over
    4 token groups. Core c handles token group c//2 with FS-half c%2;
    the two halves' partial outputs are summed on the host.
"""

import math

import numpy as np

B, S, D = 2, 512, 1024
T = B * S
E = 8
F = 768
FS = 768
TOP_K = 2
EPS = 1e-20
ROUTE_SCALE = 1.0
P = 128
N_CORES = 8
SGRP = 256        # shared-expert tokens per core (4 groups x 2 FS-halves)
FSH = FS // 2     # shared-expert intermediate slice per core
PAIR = 256        # token-group size for stage-1 matmuls (rhs free dim)

_compiled = {}


def build_nc(cap, repeat=1, act="silu"):
    """Build the per-core Bass program (same program on all 8 cores)."""
    import concourse.bacc as bacc
    import concourse.mybir as mybir
    import concourse.tile as tile

    f32 = mybir.dt.float32
    silu = mybir.ActivationFunctionType.Silu
    sigmoid = mybir.ActivationFunctionType.Sigmoid
    KD = D // P    # 8 contraction chunks
    MF = F // P    # 6 expert F-tiles
    MS = FSH // P  # 3 shared F-tiles
    NT = cap // P  # routed token tiles

    nc = bacc.Bacc("TRN2", target_bir_lowering=False, debug=False,
                   num_devices=N_CORES)

    xrT = nc.dram_tensor("xrT", [D, cap], f32, kind="ExternalInput")
    wv = nc.dram_tensor("wv", [P, NT], f32, kind="ExternalInput")
    wg_e = nc.dram_tensor("wg_e", [D, F], f32, kind="ExternalInput")
    wu_e = nc.dram_tensor("wu_e", [D, F], f32, kind="ExternalInput")
    wd_e = nc.dram_tensor("wd_e", [F, D], f32, kind="ExternalInput")
    xsT = nc.dram_tensor("xsT", [D, SGRP], f32, kind="ExternalInput")
    wg_s = nc.dram_tensor("wg_s", [D, FSH], f32, kind="ExternalInput")
    wu_s = nc.dram_tensor("wu_s", [D, FSH], f32, kind="ExternalInput")
    wd_s = nc.dram_tensor("wd_s", [FSH, D], f32, kind="ExternalInput")
    r_out = nc.dram_tensor("r_out", [cap, D], f32, kind="ExternalOutput")
    s_out = nc.dram_tensor("s_out", [SGRP, D], f32, kind="ExternalOutput")

    with tile.TileContext(nc) as tc:
        with (
            tc.tile_pool(name="wp", bufs=1) as wp,
            tc.tile_pool(name="dp", bufs=3) as dp,
            tc.tile_pool(name="pp", bufs=2, space="PSUM") as pp,
        ):
            def glu_mlp(x_dram, n_tok, wgt, wut, wdt, mf, out_dram, wv_sb):
                """out = (silu(x@Wg) * (x@Wu)) @ Wd [* w], x given as [D, n_tok]."""
                xv = x_dram.ap().rearrange("(k p) n -> p k n", p=P)
                for pr in range(n_tok // PAIR):
                    xt = dp.tile([P, KD, PAIR], f32, name="xt", tag="xt")
                    nc.sync.dma_start(out=xt[:], in_=xv[:, :, pr * PAIR:(pr + 1) * PAIR])
                    h = dp.tile([P, mf, PAIR], f32, name="h", tag="h")
                    for m in range(mf):
                        G = pp.tile([P, PAIR], f32, name="G", tag="G")
                        U = pp.tile([P, PAIR], f32, name="U", tag="U")
                        for kc in range(KD):
                            nc.tensor.matmul(G[:], wgt[:, kc, m * P:(m + 1) * P],
                                             xt[:, kc, :],
                                             start=(kc == 0), stop=(kc == KD - 1))
                        for kc in range(KD):
                            nc.tensor.matmul(U[:], wut[:, kc, m * P:(m + 1) * P],
                                             xt[:, kc, :],
                                             start=(kc == 0), stop=(kc == KD - 1))
                        if act == "silu":
                            nc.scalar.activation(h[:, m, :], G[:], silu)
                        else:
                            # CoreSim lacks Silu: silu(G) = G * sigmoid(G)
                            nc.scalar.activation(h[:, m, :], G[:], sigmoid)
                            nc.vector.tensor_mul(h[:, m, :], h[:, m, :], G[:])
                        nc.vector.tensor_mul(h[:, m, :], h[:, m, :], U[:])
                    for half in range(PAIR // P):
                        tt = pr * (PAIR // P) + half
                        for di in range(D // 512):
                            O = pp.tile([P, 512], f32, name="O", tag="O")
                            for m in range(mf):
                                nc.tensor.matmul(O[:], h[:, m, half * P:(half + 1) * P],
                                                 wdt[:, m, di * 512:(di + 1) * 512],
                                                 start=(m == 0), stop=(m == mf - 1))
                            ot = dp.tile([P, 512], f32, name="ot", tag="ot")
                            if wv_sb is not None:
                                nc.vector.tensor_scalar_mul(ot[:], O[:], wv_sb[:, tt:tt + 1])
                            else:
                                nc.vector.tensor_copy(ot[:], O[:])
                            nc.sync.dma_start(
                                out=out_dram[tt * P:(tt + 1) * P, di * 512:(di + 1) * 512],
                                in_=ot[:])

            for _rep in range(repeat):
                wg_sb = wp.tile([P, KD, F], f32, name="wg_sb")
                nc.sync.dma_start(out=wg_sb[:], in_=wg_e.ap().rearrange("(k p) f -> p k f", p=P))
                wu_sb = wp.tile([P, KD, F], f32, name="wu_sb")
                nc.sync.dma_start(out=wu_sb[:], in_=wu_e.ap().rearrange("(k p) f -> p k f", p=P))
                wd_sb = wp.tile([P, MF, D], f32, name="wd_sb")
                nc.sync.dma_start(out=wd_sb[:], in_=wd_e.ap().rearrange("(m p) d -> p m d", p=P))
                wgs_sb = wp.tile([P, KD, FSH], f32, name="wgs_sb")
                nc.sync.dma_start(out=wgs_sb[:], in_=wg_s.ap().rearrange("(k p) f -> p k f", p=P))
                wus_sb = wp.tile([P, KD, FSH], f32, name="wus_sb")
                nc.sync.dma_start(out=wus_sb[:], in_=wu_s.ap().rearrange("(k p) f -> p k f", p=P))
                wds_sb = wp.tile([P, MS, D], f32, name="wds_sb")
                nc.sync.dma_start(out=wds_sb[:], in_=wd_s.ap().rearrange("(m p) d -> p m d", p=P))
                wv_sb = wp.tile([P, NT], f32, name="wv_sb")
                nc.sync.dma_start(out=wv_sb[:], in_=wv[:])

                glu_mlp(xrT, cap, wg_sb, wu_sb, wd_sb, MF, r_out, wv_sb)
                glu_mlp(xsT, SGRP, wgs_sb, wus_sb, wds_sb, MS, s_out, None)

    nc.compile()
    return nc


def _route(x, Wr, bias):
    """Replicate the reference router numerics (jax on CPU)."""
    import jax
    import jax.numpy as jnp

    cpu = jax.devices("cpu")[0]
    with jax.default_device(cpu):
        xj = jax.device_put(np.asarray(x, np.float32), cpu)
        Wj = jax.device_put(np.asarray(Wr, np.float32), cpu)
        bj = jax.device_put(np.asarray(bias, np.float32), cpu)
        logits = xj @ Wj
        scores = jax.nn.sigmoid(logits.astype(jnp.float32))
        _, sel = jax.lax.top_k(scores + bj, TOP_K)
        top = jnp.take_along_axis(scores, sel, axis=1)
        top = top / (top.sum(-1, keepdims=True) + EPS)
        top = top * ROUTE_SCALE
        return np.asarray(sel), np.asarray(top, np.float32)


def prepare(hidden_states, W_gate_router, expert_bias, Wg, Wu, Wd, Wg_s, Wu_s, Wd_s):
    """Host-side routing + sharding. Returns (cap, in_maps, combine_fn)."""
    x = np.ascontiguousarray(np.asarray(hidden_states, np.float32).reshape(T, D))
    sel, wts = _route(x, W_gate_router, expert_bias)

    tok = np.repeat(np.arange(T), TOP_K)
    expf = np.asarray(sel).reshape(-1)
    wf = np.asarray(wts).reshape(-1)
    counts = np.bincount(expf, minlength=E)
    cap = max(PAIR, int(math.ceil(counts.max() / PAIR)) * PAIR)

    order = np.argsort(expf, kind="stable")
    starts = np.zeros(E + 1, np.int64)
    starts[1:] = np.cumsum(counts)

    in_maps = []
    toklists = []
    Wg = np.asarray(Wg, np.float32)
    Wu = np.asarray(Wu, np.float32)
    Wd = np.asarray(Wd, np.float32)
    Wg_s = np.asarray(Wg_s, np.float32)
    Wu_s = np.asarray(Wu_s, np.float32)
    Wd_s = np.asarray(Wd_s, np.float32)
    for c in range(N_CORES):
        g, hh = divmod(c, 2)
        sl = order[starts[c]:starts[c + 1]]
        n_c = counts[c]
        xr = np.zeros((cap, D), np.float32)
        xr[:n_c] = x[tok[sl]]
        wvec = np.zeros((cap,), np.float32)
        wvec[:n_c] = wf[sl]
        toklists.append(tok[sl])
        in_maps.append({
            "xrT": np.ascontiguousarray(xr.T),
            "wv": np.ascontiguousarray(wvec.reshape(cap // P, P).T),
            "wg_e": Wg[c],
            "wu_e": Wu[c],
            "wd_e": Wd[c],
            "xsT": np.ascontiguousarray(x[g * SGRP:(g + 1) * SGRP].T),
            "wg_s": np.ascontiguousarray(Wg_s[:, hh * FSH:(hh + 1) * FSH]),
            "wu_s": np.ascontiguousarray(Wu_s[:, hh * FSH:(hh + 1) * FSH]),
            "wd_s": np.ascontiguousarray(Wd_s[hh * FSH:(hh + 1) * FSH, :]),
        })

    def combine(results):
        out = np.zeros((T, D), np.float32)
        for c in range(N_CORES):
            g, hh = divmod(c, 2)
            out[g * SGRP:(g + 1) * SGRP] += results[c]["s_out"]
            n_c = counts[c]
            if n_c:
                out[toklists[c]] += results[c]["r_out"][:n_c]
        return out.reshape(B, S, D)

    return cap, in_maps, combine


def kernel(hidden_states, W_gate_router, expert_bias, Wg, Wu, Wd, Wg_s, Wu_s, Wd_s):
    from concourse.bass_utils import run_bass_kernel_spmd

    cap, in_maps, combine = prepare(hidden_states, W_gate_router, expert_bias,
                                    Wg, Wu, Wd, Wg_s, Wu_s, Wd_s)
    nc = _compiled.get(cap)
    if nc is None:
        nc = build_nc(cap)
        _compiled[cap] = nc
    res = run_bass_kernel_spmd(nc, in_maps, core_ids=list(range(N_CORES)))
    out = combine(res.results)
    return out.astype(np.asarray(hidden_states).dtype)
